# revision 18
# baseline (speedup 1.0000x reference)
"""Multi-head GAT layer (PyG-style) as a Trainium2 Bass kernel, 8-way SPMD.

Strategy (dst-sharded graph parallel):
  - Nodes sharded across 8 cores by dst ownership (6250 each). Per-core table
    rows are ROTATED so core-local nodes sit at rows [0, 6250) on every core
    (SPMD needs identical instruction streams; per-core data differs).
  - Phase 1 (projection): each core computes xp_ext = x @ [lin_w_perm|B_i|B_j]
    for ALL nodes into a DRAM table (row = [xp f16 (c-major) | ai f16 | aj
    f16]). B_i/B_j fold att into lin_w (host-side parameter preprocessing).
    xp columns are PERMUTED to channel-major (col c*8+h = head h, chan c) so
    the per-edge message scaling runs in the DVE 2x packed mode.
  - Phase 2 (edge pass): edges sorted by dst block; per 128-node block two
    dma_gathers (table split in halves for int16 indices) pull source rows,
    round-robined over 4 SWDGE queues (4 queues ~2.3x faster than 1).
    One-hots: S (edge-major) built by DVE is_equal; S^T (node-major) built
    DIRECTLY via a partition-broadcast DMA of the dst row + a 4x-mode
    tensor_scalar is_equal against a per-partition iota - no PE transposes,
    no PSUM->SBUF copies. a_i[dst] per edge = matmul(lhsT=S^T, rhs=ai_blk).
    Softmax is unnormalized (shift-invariant; eps matches the reference
    denominator). Messages and exp factors feed ONE accumulating matmul per
    tile with rhs = [xw | ex] (264 cols) producing acc and den together.
  - Finalize per block: normalize by den, LayerNorm, ELU, residual.
Padding edges point their one-hot at column 128 / -1 (matches nothing) so
they contribute exactly zero everywhere.
"""

import numpy as np

# ---- problem constants (hardcoded per spec) ----
N_NODES = 50000
N_EDGES = 800000
IN_CH = 256
HEADS = 8
HEAD_DIM = 32
HC = HEADS * HEAD_DIM  # 256
NEG_SLOPE = 0.2
LN_EPS = 1e-5
SOFTMAX_EPS = 1e-16
M_CORES = 8

P = 128
RW = 384          # table row width in f16 elems (768B): [256 xp | 8 ai | 8 aj | 112 pad]
AI_OFF = 256
AJ_OFF = 264
PROJ_W = 272      # projection output cols: 256 xp + 8 ai + 8 aj
import os
N_QUEUES = int(os.environ.get("GAT_NQ", "4"))
DMA_SCRATCH = int(os.environ.get("GAT_SCRATCH", "16384"))

# channel-major permutation of the 256 xp columns: new col c*8+h = old h*32+c
PERM_XP = np.array([h * 32 + c for c in range(32) for h in range(8)],
                   dtype=np.int64)
INV_PERM_XP = np.argsort(PERM_XP)


def _ceil_div(a, b):
    return (a + b - 1) // b


class Plan:
    """Host-side preprocessing: shapes + per-core arrays."""

    def __init__(self, x, edge_index, lin_w, att, ln_w, ln_b,
                 n_nodes=N_NODES, n_cores=M_CORES):
        N = n_nodes
        shard = N // n_cores
        assert shard * n_cores == N
        nb = _ceil_div(shard, P)              # node blocks per core
        shard_pad = nb * P
        # table rows: multiple of 1024 for clean proj chunks of 8 tiles
        tbl = _ceil_div(N, 1024) * 1024
        if tbl < N + 1:
            tbl += 1024
        half = (tbl // 2 // P) * P
        assert half <= 32767 and tbl - half <= 32767
        self.N, self.n_cores, self.shard, self.nb = N, n_cores, shard, nb
        self.shard_pad, self.tbl, self.half = shard_pad, tbl, half
        self.n_proj_tiles = tbl // P

        src = np.asarray(edge_index[0], dtype=np.int64)
        dst = np.asarray(edge_index[1], dtype=np.int64)

        # fold att into projection: B_i[c,h] = sum_k lin_w[c, h*32+k] * att_i[h,k]
        lw = np.asarray(lin_w, dtype=np.float32)
        at = np.asarray(att, dtype=np.float32)
        lw3 = lw.reshape(IN_CH, HEADS, HEAD_DIM)
        b_i = np.einsum("chk,hk->ch", lw3, at[:, :HEAD_DIM])
        b_j = np.einsum("chk,hk->ch", lw3, at[:, HEAD_DIM:])
        w_ext = np.concatenate([lw[:, PERM_XP], b_i, b_j], axis=1)  # [256,272]
        self.w_ext_f16 = w_ext.astype(np.float16)

        xf = np.asarray(x, dtype=np.float32)
        xT = np.ascontiguousarray(xf.T)  # [256, N]

        # per-core edge partition, sorted by (block, half-group)
        per_core = []
        cnt_lo = np.zeros((n_cores, nb), np.int64)
        cnt_hi = np.zeros((n_cores, nb), np.int64)
        for c in range(n_cores):
            sel = (dst // shard) == c
            s_c = src[sel]
            d_c = dst[sel] - c * shard
            srot = (s_c - c * shard) % N
            blk = d_c // P
            grp = (srot >= half).astype(np.int64)
            order = np.lexsort((grp, blk))
            s_c, d_c, srot, blk, grp = (a[order] for a in (s_c, d_c, srot, blk, grp))
            per_core.append((srot, d_c % P, blk, grp))
            for b in range(nb):
                m = blk == b
                cnt_lo[c, b] = int(np.sum(m & (grp == 0)))
                cnt_hi[c, b] = int(np.sum(m & (grp == 1)))
        self.t_lo = int(_ceil_div(int(cnt_lo.max()), P)) if cnt_lo.max() > 0 else 0
        self.t_hi = int(_ceil_div(int(cnt_hi.max()), P)) if cnt_hi.max() > 0 else 0
        self.t_tot = self.t_lo + self.t_hi
        t_lo, t_hi, t_tot = self.t_lo, self.t_hi, self.t_tot

        # per-core arrays
        self.in_maps = []
        lnw_p = np.asarray(ln_w, np.float16)[PERM_XP]
        lnb_p = np.asarray(ln_b, np.float16)[PERM_XP]
        lnw_mat = np.ascontiguousarray(np.broadcast_to(lnw_p, (P, HC)))
        lnb_mat = np.ascontiguousarray(np.broadcast_to(lnb_p, (P, HC)))
        iota_row_np = np.broadcast_to(
            np.arange(P, dtype=np.float16), (P, P)).copy()
        iota_colT = np.ascontiguousarray(np.broadcast_to(
            np.arange(P, dtype=np.float16).reshape(P, 1), (P, t_tot * P)))
        for c in range(n_cores):
            srot, dloc, blk, grp = per_core[c]
            idx16 = np.full((16, 8 * t_tot * nb), -1, np.int16)
            cnts = np.zeros((1, 2 * nb), np.int32)
            dstc = np.full((P, t_tot * nb), P, np.float16)  # pad -> 128
            dstT = np.full((nb, t_tot * P), -1.0, np.float16)
            for b in range(nb):
                m = blk == b
                for g, toff, tcnt in ((0, 0, t_lo), (1, t_lo, t_hi)):
                    if tcnt == 0:
                        continue
                    mg = m & (grp == g)
                    rel = srot[mg] - (half if g else 0)
                    dl = dloc[mg]
                    n = rel.shape[0]
                    cap = tcnt * P
                    assert n <= cap
                    relp = np.full(cap, -1, np.int64)
                    relp[:n] = rel
                    if n == 0:
                        relp[0] = 0
                        n = 1
                    cnts[0, 2 * b + g] = n
                    dlp = np.full(cap, P, np.int64)
                    dlp[:n] = dl
                    # idx layout: index i of this gather -> [i%16, gcol0 + i//16]
                    gcol0 = 8 * (b * t_tot + toff)
                    idx16[:, gcol0:gcol0 + 8 * tcnt] = (
                        relp.astype(np.int16).reshape(-1, 16).T)
                    # dst slots: edge i -> [i%128, (b*t_tot+toff) + i//128]
                    dstc[:, b * t_tot + toff: b * t_tot + toff + tcnt] = (
                        dlp.astype(np.float16).reshape(-1, P).T)
                    # dstT: edge i -> col (toff + i//128)*128 + i%128
                    dT = dlp.astype(np.float16)
                    dT[n:] = -1.0
                    dstT[b, toff * P: (toff + tcnt) * P] = dT
            idx_full = np.tile(idx16, (8, 1))  # replicate across Q7 cores

            xr = np.roll(xT, -c * shard, axis=1)
            xT_rot = np.zeros((IN_CH, self.tbl), np.float16)
            xT_rot[:, :N] = xr.astype(np.float16)

            x_res = np.zeros((shard_pad, HC), np.float16)
            x_res[:shard] = (xf[c * shard:(c + 1) * shard][:, PERM_XP]
                             - 1.0).astype(np.float16)

            dstT_pad = np.zeros((P, t_tot * P), np.float16)
            dstT_pad[:nb] = dstT

            self.in_maps.append({
                "xT": xT_rot,
                "w_ext": self.w_ext_f16,
                "idx": idx_full,
                "cnts": cnts,
                "dstc": dstc,
                "dstT": dstT_pad,
                "x_res": x_res,
                "lnw_mat": lnw_mat,
                "lnb_mat": lnb_mat,
                "iota_row": iota_row_np,
                "iota_colT": iota_colT,
            })

    def cache_key(self):
        return (self.N, self.n_cores, self.t_lo, self.t_hi)


def build_nc(plan):
    import concourse.bass as bass
    import concourse.bacc as bacc
    import concourse.mybir as mybir
    import concourse.tile as tile
    from concourse import library_config

    fp16 = mybir.dt.float16
    fp32 = mybir.dt.float32
    i16 = mybir.dt.int16
    Alu = mybir.AluOpType
    Act = mybir.ActivationFunctionType

    NB, TBL, HALF = plan.nb, plan.tbl, plan.half
    T_LO, T_HI, T_TOT = plan.t_lo, plan.t_hi, plan.t_tot
    SHARD_PAD = plan.shard_pad
    NPT = plan.n_proj_tiles  # projection tiles (TBL/128)
    CHUNK = 16               # proj tiles per xT load chunk
    WGRP = 8                 # proj tiles per table write

    nc = bacc.Bacc(None, target_bir_lowering=False, debug=False,
                   num_swdge_queues=N_QUEUES,
                   dynamic_dma_scratch_size=DMA_SCRATCH)

    xT = nc.dram_tensor("xT", [IN_CH, TBL], fp16, kind="ExternalInput")
    w_ext = nc.dram_tensor("w_ext", [IN_CH, PROJ_W], fp16, kind="ExternalInput")
    idx = nc.dram_tensor("idx", [P, 8 * T_TOT * NB], i16, kind="ExternalInput")
    cnts = nc.dram_tensor("cnts", [1, 2 * NB], mybir.dt.int32,
                          kind="ExternalInput")
    dstc = nc.dram_tensor("dstc", [P, T_TOT * NB], fp16, kind="ExternalInput")
    dstT = nc.dram_tensor("dstT", [P, T_TOT * P], fp16, kind="ExternalInput")
    x_res = nc.dram_tensor("x_res", [SHARD_PAD, HC], fp16, kind="ExternalInput")
    lnw_mat = nc.dram_tensor("lnw_mat", [P, HC], fp16, kind="ExternalInput")
    lnb_mat = nc.dram_tensor("lnb_mat", [P, HC], fp16, kind="ExternalInput")
    iota_row_t = nc.dram_tensor("iota_row", [P, P], fp16, kind="ExternalInput")
    iota_colT_t = nc.dram_tensor("iota_colT", [P, T_TOT * P], fp16,
                                 kind="ExternalInput")
    out = nc.dram_tensor("out", [SHARD_PAD, HC], fp16, kind="ExternalOutput")

    table = nc.dram_tensor("table", [TBL, RW], fp16)

    with tile.TileContext(nc) as tc:
        with tc.tile_pool(name="const", bufs=1) as cpool:
            # ---- constants ----
            iota_row = cpool.tile([P, P], fp16)
            nc.sync.dma_start(iota_row[:], iota_row_t[:])
            iota_colT = cpool.tile([P, T_TOT * P], fp16)
            nc.sync.dma_start(iota_colT[:], iota_colT_t[:])
            wk = cpool.tile([P, 2, PROJ_W], fp16)
            nc.sync.dma_start(wk[:], w_ext[:].rearrange("(k p) w -> p k w", p=P))
            lnw = cpool.tile([P, HC], fp16)
            nc.sync.dma_start(lnw[:], lnw_mat[:])
            lnb = cpool.tile([P, HC], fp16)
            nc.sync.dma_start(lnb[:], lnb_mat[:])
            eps_t = cpool.tile([P, 1], fp32)
            nc.vector.memset(eps_t[:], LN_EPS)
            zeros16 = cpool.tile([P, HC], fp16)
            nc.vector.memset(zeros16[:], 0.0)
            idx_sb = cpool.tile([P, 8 * T_TOT * NB], i16)
            nc.sync.dma_start(idx_sb[:], idx[:])
            cnts_sb = cpool.tile([P, 2 * NB], mybir.dt.int32)
            nc.sync.dma_start(cnts_sb[0:1, :], cnts[:])
            cnt_reg = nc.alloc_register(mybir.EngineType.Pool, "cnt_reg")
            dst_sb = cpool.tile([P, T_TOT * NB], fp16)
            nc.sync.dma_start(dst_sb[:], dstc[:])

            nc.gpsimd.load_library(library_config.mlp)

            sbg_scope = tc.tile_pool(name="sb_gath", bufs=3)
            sbg_pre = sbg_scope.__enter__()

            # ---- phase 1: projection into table ----
            phase1_scope = (
                tc.tile_pool(name="psum_p", bufs=6, space="PSUM"),
                tc.tile_pool(name="sb_proj", bufs=3),
            )
            psp, sbp = (phase1_scope[0].__enter__(), phase1_scope[1].__enter__())
            n_chunks = _ceil_div(NPT, CHUNK)
            assert NPT % WGRP == 0 and CHUNK % WGRP == 0
            for ch in range(n_chunks):
                t0 = ch * CHUNK
                nt = min(CHUNK, NPT - t0)
                xa = sbp.tile([P, CHUNK * P], fp16, tag="xa")
                xb = sbp.tile([P, CHUNK * P], fp16, tag="xb")
                nc.sync.dma_start(xa[:, :nt * P], xT[0:P, t0 * P:(t0 + nt) * P])
                nc.sync.dma_start(xb[:, :nt * P], xT[P:2 * P, t0 * P:(t0 + nt) * P])
                for w0 in range(0, nt, WGRP):
                    xpa = sbp.tile([P, WGRP, PROJ_W], fp16, tag="xpa")
                    for i in range(w0, w0 + WGRP):
                        pp = psp.tile([P, PROJ_W], fp32, tag="pp")
                        nc.tensor.matmul(pp[:], lhsT=xa[:, i * P:(i + 1) * P],
                                         rhs=wk[:, 0, :], start=True, stop=False)
                        nc.tensor.matmul(pp[:], lhsT=xb[:, i * P:(i + 1) * P],
                                         rhs=wk[:, 1, :], start=False, stop=True)
                        if i % 2 == 0:
                            nc.vector.tensor_copy(xpa[:, i - w0, :], pp[:])
                        else:
                            nc.scalar.copy(xpa[:, i - w0, :], pp[:])
                    rows = slice((t0 + w0) * P, (t0 + w0 + WGRP) * P)
                    nc.sync.dma_start(
                        table[rows, 0:PROJ_W].rearrange(
                            "(g p) w -> p g w", p=P),
                        xpa[:])

            phase1_scope[1].__exit__(None, None, None)
            phase1_scope[0].__exit__(None, None, None)

            # pre-zero the gather slots while phase 1 runs (runtime-count
            # gathers leave padding tails untouched; first uses need finite
            # data). The pool rotates these same slots inside the block loop.
            for _k in range(3):
                xg_pre = sbg_pre.tile([P, T_TOT, RW], fp16, tag="xg")
                nc.gpsimd.memset(xg_pre[:], 0.0)

            # table must be fully written before any gather reads it; the
            # custom gather's DRAM read is not dependency-tracked by Tile.
            tc.strict_bb_all_engine_barrier()

            # ---- phase 2: edge pass ----
            edge_scope = (
                tc.tile_pool(name="sb_edge", bufs=2),
                tc.tile_pool(name="sb_fin", bufs=2),
                tc.tile_pool(name="ps_acc", bufs=3, space="PSUM"),
                tc.tile_pool(name="ps_ai", bufs=2, space="PSUM"),
            )
            sbe, sbf, psa, psai = [cm.__enter__() for cm in edge_scope]
            sbg = sbg_pre

            def edge_block(b):
                """Gather + attention + accumulate + pass-1 stats for block b.

                Returns (yc0, negmu) where yc0 [P,HC] fp16 is the centered
                pre-normalization output and var lands in the group tile.
                """
                nrow0 = b * P
                ai_blk = sbe.tile([P, 8], fp16, tag="ai_blk")
                nc.sync.dma_start(ai_blk[:],
                                  table[nrow0:nrow0 + P, AI_OFF:AI_OFF + 8])
                # gathers (lo/hi table halves) round-robin over SWDGE queues;
                # runtime counts skip the padding tail (idx pad = -1). Slots
                # past the count keep the previous block's rows - finite data
                # masked to zero by the one-hots (first use is memset).
                xg = sbg.tile([P, T_TOT, RW], fp16, tag="xg")
                for g, toff, tcnt in ((0, 0, T_LO), (1, T_LO, T_HI)):
                    if tcnt == 0:
                        continue
                    src_ap = table[0:HALF, :] if g == 0 else table[HALF:TBL, :]
                    gcol0 = 8 * (b * T_TOT + toff)
                    nc.gpsimd.reg_load(cnt_reg,
                                       cnts_sb[0:1, 2 * b + g:2 * b + g + 1])
                    nc.gpsimd.dma_gather(
                        out_ap=xg[:, toff:toff + tcnt, :],
                        in_ap=src_ap,
                        idxs_ap=idx_sb[:, gcol0:gcol0 + 8 * tcnt],
                        num_idxs=tcnt * P,
                        num_idxs_reg=cnt_reg,
                        elem_size=RW,
                        single_packet=False,
                        queue_num=(2 * b + g) % N_QUEUES,
                    )
                # S (edge-major one-hot) for acc/den matmuls
                s_all = sbe.tile([P, T_TOT, P], fp16, tag="s_all")
                dslice = dst_sb[:, b * T_TOT:(b + 1) * T_TOT]
                iap = iota_row[:]
                iota_b = bass.AP(iap.tensor, iap.offset,
                                 [iap.ap[0], [0, T_TOT], iap.ap[1]])
                nc.vector.tensor_tensor(
                    out=s_all[:],
                    in0=dslice.to_broadcast([P, T_TOT, P]),
                    in1=iota_b,
                    op=Alu.is_equal,
                )
                # S^T (node-major one-hot) built directly: broadcast the dst
                # row of this block across partitions, compare to the
                # per-partition iota constant (both step-1 fp16 -> 2x mode).
                dstT_b = sbe.tile([P, T_TOT * P], fp16, tag="dstT_b")
                src_row = dstT[b:b + 1, :].to_broadcast([P, T_TOT * P])
                nc.sync.dma_start(dstT_b[:], src_row)
                st_all = sbe.tile([P, T_TOT, P], fp16, tag="st_all")
                nc.vector.tensor_tensor(
                    out=st_all[:],
                    in0=dstT_b[:].rearrange("p (t e) -> p t e", t=T_TOT),
                    in1=iota_colT[:].rearrange("p (t e) -> p t e", t=T_TOT),
                    op=Alu.is_equal,
                )
                # per-edge a_i via node-major one-hots (no transposes)
                ai_ps = psai.tile([P, T_TOT, 8], fp32, tag="ai_ps")
                for t in range(T_TOT):
                    nc.tensor.matmul(ai_ps[:, t, :], lhsT=st_all[:, t, :],
                                     rhs=ai_blk[:], start=True, stop=True)
                # alpha / lrelu / exp for the whole block
                xe = sbe.tile([P, T_TOT, 264], fp16, tag="xe")
                al = sbe.tile([P, T_TOT, 8], fp32, tag="al")
                nc.vector.tensor_tensor(
                    out=al[:], in0=ai_ps[:],
                    in1=xg[:, :, AJ_OFF:AJ_OFF + 8],
                    op=Alu.add)
                nc.vector.scalar_tensor_tensor(
                    out=al[:], in0=al[:], scalar=NEG_SLOPE, in1=al[:],
                    op0=Alu.mult, op1=Alu.max)
                nc.scalar.activation(xe[:, :, 256:264], al[:], Act.Exp)
                # weighted messages into xe[:, :, 0:256] (c-major: 2x DVE)
                exap = xe[:, :, 256:264]
                ex_b = bass.AP(exap.tensor, exap.offset,
                               [exap.ap[0], exap.ap[1], [0, HEAD_DIM],
                                exap.ap[2]])
                nc.vector.tensor_tensor(
                    out=xe[:, :, 0:HC].rearrange("p t (c h) -> p t c h",
                                                 h=HEADS),
                    in0=xg[:, :, 0:HC].rearrange("p t (c h) -> p t c h",
                                                 h=HEADS),
                    in1=ex_b,
                    op=Alu.mult)
                # acc+den in one accumulating matmul per tile
                accden = psa.tile([P, 264], fp32, tag="accden")
                for t in range(T_TOT):
                    nc.tensor.matmul(accden[:], lhsT=s_all[:, t, :],
                                     rhs=xe[:, t, :],
                                     start=(t == 0), stop=(t == T_TOT - 1))
                return accden

            def pass1(b, accden, y0h, mus_col, sqs_col):
                """Softmax-normalize; y0h = y0 fp16, plus sum and sum-sq."""
                acc = accden[:, 0:HC]
                den = accden[:, HC:HC + 8]
                d8 = sbf.tile([P, 8], fp32, tag="d8")
                nc.vector.tensor_scalar_add(d8[:], den, SOFTMAX_EPS)
                r8 = sbf.tile([P, 8], fp32, tag="r8")
                nc.vector.reciprocal(r8[:], d8[:])
                r8ap = r8[:]
                r8_b = bass.AP(r8ap.tensor, r8ap.offset,
                               [r8ap.ap[0], [0, HEAD_DIM], r8ap.ap[1]])
                nc.vector.scalar_tensor_tensor(
                    out=y0h.rearrange("p (c h) -> p c h", h=HEADS),
                    in0=acc.rearrange("p (c h) -> p c h", h=HEADS),
                    scalar=1.0,
                    in1=r8_b,
                    op0=Alu.mult, op1=Alu.mult,
                    accum_out=mus_col)
                sq = sbf.tile([P, HC], fp16, tag=f"sq{b % 2}")
                nc.vector.scalar_tensor_tensor(
                    out=sq[:], in0=y0h, scalar=1.0, in1=y0h,
                    op0=Alu.mult, op1=Alu.mult, accum_out=sqs_col)

            def pass2(b, y0h, rstd_col, nmrs_col):
                """yc = y0*rstd*lnw + (lnb - mu*rstd*lnw); ELU; residual."""
                nrow0 = b * P
                bt = sbf.tile([P, HC], fp16, tag="bt")
                nc.vector.scalar_tensor_tensor(
                    out=bt[:], in0=lnw[:], scalar=nmrs_col,
                    in1=lnb[:], op0=Alu.mult, op1=Alu.add)
                yc = sbf.tile([P, HC], fp16, tag="yc")
                nc.vector.scalar_tensor_tensor(
                    out=yc[:], in0=y0h, scalar=rstd_col,
                    in1=lnw[:], op0=Alu.mult, op1=Alu.mult)
                nc.vector.tensor_tensor(out=yc[:], in0=yc[:], in1=bt[:],
                                        op=Alu.add)
                mneg = sbf.tile([P, HC], fp16, tag="mneg")
                nc.vector.scalar_tensor_tensor(
                    out=mneg[:], in0=yc[:], scalar=0.0, in1=zeros16[:],
                    op0=Alu.min, op1=Alu.min)
                ee = sbf.tile([P, HC], fp16, tag="ee")
                nc.scalar.activation(ee[:], mneg[:], Act.Exp)
                xr = sbf.tile([P, HC], fp16, tag="xr")
                nc.sync.dma_start(xr[:], x_res[nrow0:nrow0 + P, :])
                fin = sbf.tile([P, HC], fp16, tag="fin")
                nc.vector.scalar_tensor_tensor(
                    out=fin[:], in0=yc[:], scalar=0.0, in1=ee[:],
                    op0=Alu.max, op1=Alu.add)
                nc.vector.tensor_tensor(out=fin[:], in0=fin[:], in1=xr[:],
                                        op=Alu.add)
                nc.sync.dma_start(out[nrow0:nrow0 + P, :], fin[:])

            GF = 13  # blocks per finalize group (one Sqrt per group)
            b0 = 0
            gi = 0
            while b0 < NB:
                gn = min(GF, NB - b0)
                y0_g = sbf.tile([P, GF, HC], fp16, tag=f"y0_{gi % 2}")
                mus_g = sbf.tile([P, GF], fp32, tag=f"mus_{gi % 2}")
                sqs_g = sbf.tile([P, GF], fp32, tag=f"sqs_{gi % 2}")
                for j in range(gn):
                    accden = edge_block(b0 + j)
                    pass1(b0 + j, accden, y0_g[:, j, :],
                          mus_g[:, j:j + 1], sqs_g[:, j:j + 1])
                # group stats: mu, var = sqs/HC - mu^2, rstd, -mu*rstd
                mu_g = sbf.tile([P, GF], fp32, tag=f"mu_{gi % 2}")
                nc.vector.tensor_scalar_mul(mu_g[:, :gn], mus_g[:, :gn],
                                            1.0 / HC)
                mu2_g = sbf.tile([P, GF], fp32, tag=f"mu2_{gi % 2}")
                nc.vector.scalar_tensor_tensor(
                    out=mu2_g[:, :gn], in0=mu_g[:, :gn], scalar=1.0,
                    in1=mu_g[:, :gn], op0=Alu.mult, op1=Alu.mult)
                rv_g = sbf.tile([P, GF], fp32, tag=f"rv_{gi % 2}")
                nc.vector.scalar_tensor_tensor(
                    out=rv_g[:, :gn], in0=sqs_g[:, :gn], scalar=1.0 / HC,
                    in1=mu2_g[:, :gn], op0=Alu.mult, op1=Alu.subtract)
                sdv_g = sbf.tile([P, GF], fp32, tag=f"sdv_{gi % 2}")
                nc.scalar.activation(sdv_g[:, :gn], rv_g[:, :gn], Act.Sqrt,
                                     bias=eps_t[:, 0:1], scale=1.0)
                rstd_g = sbf.tile([P, GF], fp32, tag=f"rstd_{gi % 2}")
                nc.vector.reciprocal(rstd_g[:, :gn], sdv_g[:, :gn])
                nmrs_g = sbf.tile([P, GF], fp32, tag=f"nmrs_{gi % 2}")
                nc.vector.scalar_tensor_tensor(
                    out=nmrs_g[:, :gn], in0=mu_g[:, :gn], scalar=-1.0,
                    in1=rstd_g[:, :gn], op0=Alu.mult, op1=Alu.mult)
                for j in range(gn):
                    pass2(b0 + j, y0_g[:, j, :], rstd_g[:, j:j + 1],
                          nmrs_g[:, j:j + 1])
                b0 += gn
                gi += 1

            for cm in reversed(edge_scope):
                cm.__exit__(None, None, None)
            sbg_scope.__exit__(None, None, None)

    nc.compile()
    return nc


_NC_CACHE = {}


def _run(plan, trace=False):
    from concourse.bass_utils import run_bass_kernel_spmd
    key = plan.cache_key()
    if key not in _NC_CACHE:
        _NC_CACHE[key] = build_nc(plan)
    nc = _NC_CACHE[key]
    r = run_bass_kernel_spmd(nc, plan.in_maps,
                             core_ids=list(range(plan.n_cores)), trace=trace)
    outs = [res["out"][:plan.shard] for res in r.results]
    full = np.concatenate(outs, axis=0).astype(np.float32)
    return full[:, INV_PERM_XP], r


def kernel(x, edge_index, lin_w, att, ln_w, ln_b):
    plan = Plan(x, edge_index, lin_w, att, ln_w, ln_b)
    out, _ = _run(plan)
    return np.ascontiguousarray(out, dtype=np.float32)


# ---------------- self-contained mini test ----------------
def _np_reference(x, edge_index, lin_w, att, ln_w, ln_b):
    N = x.shape[0]
    src, dst = edge_index[0], edge_index[1]
    xp = (x @ lin_w).reshape(N, HEADS, HEAD_DIM)
    a_i = np.einsum("nhc,hc->nh", xp, att[:, :HEAD_DIM])
    a_j = np.einsum("nhc,hc->nh", xp, att[:, HEAD_DIM:])
    alpha = a_i[dst] + a_j[src]
    alpha = np.where(alpha >= 0, alpha, NEG_SLOPE * alpha)
    amax = np.full((N, HEADS), -np.inf, np.float32)
    np.maximum.at(amax, dst, alpha)
    amax = np.where(np.isfinite(amax), amax, 0.0)
    ex = np.exp(alpha - amax[dst])
    denom = np.zeros((N, HEADS), np.float32)
    np.add.at(denom, dst, ex)
    alpha = ex / (denom[dst] + SOFTMAX_EPS)
    msg = xp[src] * alpha[:, :, None]
    out = np.zeros((N, HEADS, HEAD_DIM), np.float32)
    np.add.at(out, dst, msg)
    out = out.reshape(N, HC)
    mu = out.mean(-1, keepdims=True)
    var = ((out - mu) ** 2).mean(-1, keepdims=True)
    out = (out - mu) / np.sqrt(var + LN_EPS) * ln_w + ln_b
    out = np.where(out > 0, out, np.exp(np.minimum(out, 0)) - 1)
    return out + x


if __name__ == "__main__":
    import sys, time
    mini_n = int(sys.argv[1]) if len(sys.argv) > 1 else 1024
    mini_e = int(sys.argv[2]) if len(sys.argv) > 2 else 8192
    rng = np.random.default_rng(0)
    x = rng.standard_normal((mini_n, IN_CH), dtype=np.float32)
    ei = rng.integers(0, mini_n, (2, mini_e)).astype(np.int64)
    lw = (rng.standard_normal((IN_CH, HC), dtype=np.float32) / 16.0)
    at = rng.standard_normal((HEADS, 2 * HEAD_DIM), dtype=np.float32) * 0.1
    lnw = np.ones(HC, np.float32)
    lnb = np.zeros(HC, np.float32)

    t0 = time.time()
    plan = Plan(x, ei, lw, at, lnw, lnb, n_nodes=mini_n)
    print(f"plan: t_lo={plan.t_lo} t_hi={plan.t_hi} nb={plan.nb} "
          f"tbl={plan.tbl} half={plan.half} prep={time.time()-t0:.1f}s")
    t0 = time.time()
    got, r = _run(plan, trace="--trace" in sys.argv)
    print(f"run: {time.time()-t0:.1f}s  exec={r.exec_time_ns}")
    want = _np_reference(x, ei, lw, at, lnw, lnb)
    err = np.abs(got - want)
    rel = err.max() / np.abs(want).max()
    print(f"abs err {err.max():.3e}  rel(absmax) {rel:.3e}")
